# revision 4
# baseline (speedup 1.0000x reference)
"""Trainium2 Bass kernel: 16-member MLP ensemble (1024 -> 256 relu -> 128 relu -> 16 tanh).

Sharding: expert-parallel over the ensemble axis -- 2 members per NeuronCore x 8 cores,
fully independent (no collectives).

Device layout strategy: the PE contracts along the partition dim, so every operand is
pre-arranged host-side with the contraction dim on partitions:
  x   -> [mpc, 128, 8, B]   (x^T tiles: input-feature chunks on partitions)
  W1  -> [mpc, 128, 8, 256] (W1^T: lhsT tiles [K=128, M=256])
  W2  -> [mpc, 128, 2, 128]
  W3  -> [mpc, 128, 16]
Hidden activations stay in SBUF between layers (h1/h2 never touch HBM); the kernel output
is out^T [mpc, 16, B] per core, un-transposed on host.

Matmuls run as float32r (fp32 data, full-rate PE mode for moving-dim >= 256).
"""

import numpy as np

import concourse.bacc as bacc
import concourse.bass as bass
import concourse.mybir as mybir
import concourse.tile as tile
from concourse.bass_utils import run_bass_kernel_spmd

M, B, Z = 16, 4096, 16
N_CORES = 8
MPC = M // N_CORES          # models per core
D_IN, H1, H2 = 1024, 256, 128
BT = 512                    # batch tile (fp32 moving-operand max / one PSUM bank)
NBT = B // BT
KC1 = D_IN // 128           # contraction chunks, layer 1
KC2 = H1 // 128             # contraction chunks, layer 2
OC1 = H1 // 128             # output chunks, layer 1

F32 = mybir.dt.float32
F32R = mybir.dt.float32r
AF = mybir.ActivationFunctionType

_cached = None
last_results = None         # BassKernelResults from the most recent run (for test harness)


def build_bass():
    nc = bacc.Bacc("TRN2", target_bir_lowering=False, debug=False, num_devices=N_CORES)

    xh = nc.dram_tensor("xh", [MPC, 128, KC1, B], F32R, kind="ExternalInput")
    w1h = nc.dram_tensor("w1h", [MPC, 128, KC1, H1], F32R, kind="ExternalInput")
    b1h = nc.dram_tensor("b1h", [MPC, 128, OC1], F32, kind="ExternalInput")
    w2h = nc.dram_tensor("w2h", [MPC, 128, KC2, H2], F32R, kind="ExternalInput")
    b2h = nc.dram_tensor("b2h", [MPC, 128, 1], F32, kind="ExternalInput")
    w3h = nc.dram_tensor("w3h", [MPC, 128, Z], F32R, kind="ExternalInput")
    b3h = nc.dram_tensor("b3h", [MPC, Z, 1], F32, kind="ExternalInput")
    outh = nc.dram_tensor("outh", [MPC, Z, B], F32, kind="ExternalOutput")

    with tile.TileContext(nc) as tc:
        with (
            tc.tile_pool(name="weights", bufs=1) as wp,
            tc.tile_pool(name="xin", bufs=4) as xp,
            tc.tile_pool(name="hid", bufs=4) as hp,
            tc.tile_pool(name="outs", bufs=2) as op,
            tc.tile_pool(name="ps", bufs=2, space="PSUM") as pp,
        ):
            # Per-model weight/bias tiles (resident for the whole kernel).
            wt = []
            for m in range(MPC):
                w1 = wp.tile([128, KC1, H1], F32R, name=f"w1_{m}", tag=f"w1_{m}")
                nc.gpsimd.dma_start(w1[:], w1h[m])
                w2 = wp.tile([128, KC2, H2], F32R, name=f"w2_{m}", tag=f"w2_{m}")
                nc.gpsimd.dma_start(w2[:], w2h[m])
                w3 = wp.tile([128, Z], F32R, name=f"w3_{m}", tag=f"w3_{m}")
                nc.gpsimd.dma_start(w3[:], w3h[m])
                b1 = wp.tile([128, OC1], F32, name=f"b1_{m}", tag=f"b1_{m}")
                nc.gpsimd.dma_start(b1[:], b1h[m])
                b2 = wp.tile([128, 1], F32, name=f"b2_{m}", tag=f"b2_{m}")
                nc.gpsimd.dma_start(b2[:], b2h[m])
                b3 = wp.tile([Z, 1], F32, name=f"b3_{m}", tag=f"b3_{m}")
                nc.gpsimd.dma_start(b3[:], b3h[m])
                wt.append((w1, w2, w3, b1, b2, b3))

            # Weight-touch warmups: the walrus fp32r self-loading matmul has a
            # single sync-wait slot, so no real matmul may wait on both its
            # weight DMA and its rhs producer. Touch each weight tile with a
            # tiny matmul that carries the weight-DMA wait alone; afterwards the
            # PE clock covers every weight sem and real matmuls keep <=1 wait.
            with tc.tile_pool(name="warm", bufs=1, space="PSUM") as wpp:
                wps = wpp.tile([128, 16], F32, name="warm_ps", tag="warm_ps")
                for m in range(MPC):
                    w1, w2, w3, _, _, _ = wt[m]
                    nc.tensor.matmul(wps[:], lhsT=w1[:, 0, 0:128],
                                     rhs=w1[:, 0, 0:16], start=True, stop=True)
                    nc.tensor.matmul(wps[:], lhsT=w2[:, 0, 0:128],
                                     rhs=w2[:, 0, 0:16], start=True, stop=True)
                    nc.tensor.matmul(wps[0:16, :], lhsT=w3[:, 0:16],
                                     rhs=w3[:, 0:16], start=True, stop=True)

            for m in range(MPC):
                w1, w2, w3, b1, b2, b3 = wt[m]
                # Whole-model output staged in SBUF; one DMA out per model.
                osb = op.tile([Z, B], F32, name=f"osb_{m}", tag="osb")
                for t in range(NBT):
                    bs = slice(t * BT, (t + 1) * BT)
                    xt = xp.tile([128, KC1, BT], F32R, name=f"x_{m}_{t}", tag="xt")
                    nc.sync.dma_start(xt[:], xh[m][:, :, bs])

                    # Layer 1: h1[oc] = relu(sum_c W1T[c,oc].T @ xT[c] + b1[oc])
                    h1c = []
                    for oc in range(OC1):
                        ps1 = pp.tile([128, BT], F32, name=f"ps1_{m}_{t}_{oc}", tag="ps1")
                        for c in range(KC1):
                            nc.tensor.matmul(
                                ps1[:],
                                lhsT=w1[:, c, oc * 128:(oc + 1) * 128],
                                rhs=xt[:, c, :],
                                start=(c == 0),
                                stop=(c == KC1 - 1),
                            )
                        h1 = hp.tile([128, BT], F32R, name=f"h1_{m}_{t}_{oc}", tag="h1")
                        nc.scalar.activation(h1[:], ps1[:], AF.Relu, bias=b1[:, oc:oc + 1])
                        h1c.append(h1)

                    # Layer 2: h2 = relu(sum_c W2T[c].T @ h1[c] + b2)
                    ps2 = pp.tile([128, BT], F32, name=f"ps2_{m}_{t}", tag="ps2")
                    for c in range(KC2):
                        nc.tensor.matmul(
                            ps2[:],
                            lhsT=w2[:, c, :],
                            rhs=h1c[c][:],
                            start=(c == 0),
                            stop=(c == KC2 - 1),
                        )
                    h2 = hp.tile([128, BT], F32R, name=f"h2_{m}_{t}", tag="h2")
                    nc.scalar.activation(h2[:], ps2[:], AF.Relu, bias=b2[:, 0:1])

                    # Layer 3: out = tanh(W3T.T @ h2 + b3)
                    ps3 = pp.tile([Z, BT], F32, name=f"ps3_{m}_{t}", tag="ps3")
                    nc.tensor.matmul(
                        ps3[:],
                        lhsT=w3[:],
                        rhs=h2[:],
                        start=True,
                        stop=True,
                    )
                    nc.scalar.activation(osb[:, bs], ps3[:], AF.Tanh, bias=b3[:, 0:1])

                nc.sync.dma_start(outh[m], osb[:])

    nc.compile()
    return nc


def make_in_maps(x, W1, b1, W2, b2, W3, b3):
    """Host-side shard + layout prep. Returns one input map per core."""
    xb = np.asarray(x, dtype=np.float32).reshape(M, B, D_IN)
    W1 = np.asarray(W1, dtype=np.float32)
    W2 = np.asarray(W2, dtype=np.float32)
    W3 = np.asarray(W3, dtype=np.float32)
    b1 = np.asarray(b1, dtype=np.float32)
    b2 = np.asarray(b2, dtype=np.float32)
    b3 = np.asarray(b3, dtype=np.float32)

    in_maps = []
    for core in range(N_CORES):
        sl = slice(core * MPC, (core + 1) * MPC)
        # x: [mpc,B,1024] -> i=(c,p) -> [mpc,128,KC1,B]
        xh = np.ascontiguousarray(
            xb[sl].reshape(MPC, B, KC1, 128).transpose(0, 3, 2, 1))
        # W1: [mpc,256,1024] -> [mpc,128,KC1,256]
        w1h = np.ascontiguousarray(
            W1[sl].reshape(MPC, H1, KC1, 128).transpose(0, 3, 2, 1))
        # W2: [mpc,128,256] -> [mpc,128,KC2,128]
        w2h = np.ascontiguousarray(
            W2[sl].reshape(MPC, H2, KC2, 128).transpose(0, 3, 2, 1))
        # W3: [mpc,16,128] -> [mpc,128,16]
        w3h = np.ascontiguousarray(W3[sl].transpose(0, 2, 1))
        b1t = np.ascontiguousarray(b1[sl].reshape(MPC, OC1, 128).transpose(0, 2, 1))
        b2t = np.ascontiguousarray(b2[sl].reshape(MPC, 128, 1))
        b3t = np.ascontiguousarray(b3[sl].reshape(MPC, Z, 1))
        in_maps.append({
            "xh": xh, "w1h": w1h, "b1h": b1t,
            "w2h": w2h, "b2h": b2t, "w3h": w3h, "b3h": b3t,
        })
    return in_maps


def kernel(x, W1, b1, W2, b2, W3, b3):
    global _cached, last_results
    if _cached is None:
        _cached = build_bass()
    nc = _cached

    in_maps = make_in_maps(x, W1, b1, W2, b2, W3, b3)
    res = run_bass_kernel_spmd(nc, in_maps, list(range(N_CORES)))
    last_results = res

    # outh per core: [MPC, Z, B] -> full output [M, B, Z]
    parts = [r["outh"] for r in res.results]
    out_t = np.concatenate(parts, axis=0)             # [M, Z, B]
    return np.ascontiguousarray(out_t.transpose(0, 2, 1)).astype(np.float32)


# revision 16
# speedup vs baseline: 4.1405x; 4.1405x over previous
"""Trainium2 Bass kernel: 16-member MLP ensemble (1024 -> 256 relu -> 128 relu -> 16 tanh).

Sharding: expert-parallel over the ensemble axis -- 2 members per NeuronCore x 8 cores,
fully independent (no collectives).

Device layout strategy: the PE contracts along the partition dim, so every operand is
pre-arranged host-side with the contraction dim on partitions:
  x   -> [mpc, 128, 8, B]   (x^T tiles: input-feature chunks on partitions)
  W1  -> [mpc, 128, 8, 256] (W1^T: lhsT tiles [K=128, M=256])
  W2  -> [mpc, 128, 2, 128]
  W3  -> [mpc, 128, 16]
Hidden activations stay in SBUF between layers (h1/h2 never touch HBM); the kernel output
is out^T [mpc, 16, B] per core, un-transposed on host.

Matmuls run as float32r (fp32 data, full-rate PE mode for moving-dim >= 256).
"""

import numpy as np

import concourse.bacc as bacc
import concourse.bass as bass
import concourse.mybir as mybir
import concourse.tile as tile
from concourse.bass_utils import run_bass_kernel_spmd
from concourse.tile import add_dep_helper

M, B, Z = 16, 4096, 16
N_CORES = 8
MPC = M // N_CORES          # models per core
D_IN, H1, H2 = 1024, 256, 128
BT = 512                    # batch tile (fp32 moving-operand max / one PSUM bank)
NBT = B // BT
KC1 = D_IN // 128           # contraction chunks, layer 1
KC2 = H1 // 128             # contraction chunks, layer 2
OC1 = H1 // 128             # output chunks, layer 1

F32 = mybir.dt.float32
F32R = mybir.dt.float32r
AF = mybir.ActivationFunctionType

_cached = None
last_results = None         # BassKernelResults from the most recent run (for test harness)


def build_bass():
    nc = bacc.Bacc("TRN2", target_bir_lowering=False, debug=False, num_devices=N_CORES)

    xh = nc.dram_tensor("xh", [MPC, 128, KC1, B], F32R, kind="ExternalInput")
    w1h = nc.dram_tensor("w1h", [MPC, 128, KC1, H1], F32R, kind="ExternalInput")
    b1h = nc.dram_tensor("b1h", [MPC, 128, OC1], F32, kind="ExternalInput")
    w2h = nc.dram_tensor("w2h", [MPC, 128, KC2, H2], F32R, kind="ExternalInput")
    b2h = nc.dram_tensor("b2h", [MPC, 128, 1], F32, kind="ExternalInput")
    w3h = nc.dram_tensor("w3h", [MPC, 128, Z], F32R, kind="ExternalInput")
    b3h = nc.dram_tensor("b3h", [MPC, Z, 1], F32, kind="ExternalInput")
    outh = nc.dram_tensor("outh", [MPC, Z, B], F32, kind="ExternalOutput")

    with tile.TileContext(nc) as tc:
        with (
            tc.tile_pool(name="weights", bufs=1) as wp,
            tc.tile_pool(name="xin", bufs=5) as xp,
            tc.tile_pool(name="hid", bufs=4) as hp,
            tc.tile_pool(name="outs", bufs=4) as op,
            tc.tile_pool(name="ps", bufs=2, space="PSUM") as pp,
            tc.tile_pool(name="ps1p", bufs=3, space="PSUM") as pp1,
            tc.tile_pool(name="warm", bufs=1, space="PSUM") as wpp,
        ):
            # Weight/bias DMAs. w1 of model 0 goes first so the PE can start
            # layer 1 as early as possible; everything else trickles in behind
            # the first x tile on the queue.
            wt = [[None] * 6 for _ in range(MPC)]
            wdmas = []
            w1_0 = wp.tile([128, KC1, H1], F32R, name="w1_0", tag="w1_0")
            wdmas.append(nc.sync.dma_start(w1_0[:], w1h[0]))
            wt[0][0] = w1_0
            for m in range(MPC):
                if m > 0:
                    w1m = wp.tile([128, KC1, H1], F32R, name=f"w1_{m}", tag=f"w1_{m}")
                    wdmas.append(nc.sync.dma_start(w1m[:], w1h[m]))
                    wt[m][0] = w1m
                w2 = wp.tile([128, KC2, H2], F32R, name=f"w2_{m}", tag=f"w2_{m}")
                wdmas.append(nc.sync.dma_start(w2[:], w2h[m]))
                w3 = wp.tile([128, Z], F32R, name=f"w3_{m}", tag=f"w3_{m}")
                wdmas.append(nc.sync.dma_start(w3[:], w3h[m]))
                b1 = wp.tile([128, OC1], F32, name=f"b1_{m}", tag=f"b1_{m}")
                wdmas.append(nc.sync.dma_start(b1[:], b1h[m]))
                b2 = wp.tile([128, 1], F32, name=f"b2_{m}", tag=f"b2_{m}")
                wdmas.append(nc.sync.dma_start(b2[:], b2h[m]))
                b3 = wp.tile([Z, 1], F32, name=f"b3_{m}", tag=f"b3_{m}")
                wdmas.append(nc.sync.dma_start(b3[:], b3h[m]))
                wt[m][1:] = [w2, w3, b1, b2, b3]

            wps = wpp.tile([128, 16], F32, name="warm_ps", tag="warm_ps")

            def emit_chunk(m, tag, w1, w2, w3, b1, b2, b3, xt, xs, outs, width):
                """One fused 3-layer pass over `width` batch columns.
                xt[:, c, xs] supplies the layer-1 rhs; result stored to outh[m][:, outs]."""
                h1c = []
                for oc in range(OC1):
                    ps1 = pp1.tile([128, width], F32, name=f"ps1_{tag}_{oc}", tag="ps1")
                    for c in range(KC1):
                        nc.tensor.matmul(
                            ps1[:],
                            lhsT=w1[:, c, oc * 128:(oc + 1) * 128],
                            rhs=xt[:, c, xs],
                            start=(c == 0),
                            stop=(c == KC1 - 1),
                        )
                    h1 = hp.tile([128, width], F32R, name=f"h1_{tag}_{oc}", tag="h1")
                    nc.scalar.activation(h1[:], ps1[:], AF.Relu, bias=b1[:, oc:oc + 1])
                    h1c.append(h1)

                ps2 = pp.tile([128, width], F32, name=f"ps2_{tag}", tag="ps2")
                for c in range(KC2):
                    nc.tensor.matmul(
                        ps2[:],
                        lhsT=w2[:, c, :],
                        rhs=h1c[c][:],
                        start=(c == 0),
                        stop=(c == KC2 - 1),
                    )
                h2 = hp.tile([128, width], F32R, name=f"h2_{tag}", tag="h2")
                nc.scalar.activation(h2[:], ps2[:], AF.Relu, bias=b2[:, 0:1])

                ps3 = pp.tile([Z, width], F32, name=f"ps3_{tag}", tag="ps3")
                nc.tensor.matmul(ps3[:], lhsT=w3[:], rhs=h2[:], start=True, stop=True)
                ot = op.tile([Z, width], F32, name=f"ot_{tag}", tag="ot")
                nc.scalar.activation(ot[:], ps3[:], AF.Tanh, bias=b3[:, 0:1])
                nc.gpsimd.dma_start(outh[m][:, outs], ot[:])

            XW = BT               # columns per x DMA (2 MiB transfers)
            for m in range(MPC):
                w1, w2, w3, b1, b2, b3 = wt[m]
                # Weight-touch warmups, per model: the walrus fp32r self-loading
                # matmul has a single sync-wait slot, so no real matmul may wait
                # on both its weight DMA and its rhs producer. Touch each weight
                # tile with a tiny matmul carrying the weight-DMA wait alone.
                nc.tensor.matmul(wps[:], lhsT=w1[:, 0, 0:128],
                                 rhs=w1[:, 0, 0:16], start=True, stop=True)
                nc.tensor.matmul(wps[:], lhsT=w2[:, 0, 0:128],
                                 rhs=w2[:, 0, 0:16], start=True, stop=True)
                nc.tensor.matmul(wps[0:16, :], lhsT=w3[:, 0:16],
                                 rhs=w3[:, 0:16], start=True, stop=True)

                last = MPC - 1 == m
                for tx in range(B // XW):
                    xt = xp.tile([128, KC1, XW], F32R, name=f"x_{m}_{tx}", tag="xt")
                    x_ap = xh[m][:, :, tx * XW:(tx + 1) * XW]
                    tail = last and tx == B // XW - 1
                    if not tail:
                        xdma = nc.sync.dma_start(xt[:], x_ap)
                        if m == 0 and tx == 0:
                            # Keeps the first bulk x chunk at the queue head with
                            # the small weight DMAs immediately behind it.
                            for wd in wdmas:
                                add_dep_helper(wd.ins, xdma.ins, sync=False,
                                               reason="weights before x bulk stream")
                        emit_chunk(m, f"{m}_{tx}", w1, w2, w3, b1, b2, b3,
                                   xt, slice(0, XW), slice(tx * XW, (tx + 1) * XW), XW)
                    else:
                        # Final chunk: split into halves so the tail drain
                        # overlaps the last x bytes still in flight.
                        hw_ = XW // 2
                        for h in range(2):
                            nc.sync.dma_start(xt[:, :, h * hw_:(h + 1) * hw_],
                                              x_ap[:, :, h * hw_:(h + 1) * hw_])
                            emit_chunk(m, f"{m}_{tx}_h{h}", w1, w2, w3, b1, b2, b3,
                                       xt, slice(h * hw_, (h + 1) * hw_),
                                       slice(tx * XW + h * hw_, tx * XW + (h + 1) * hw_), hw_)

    nc.compile()
    return nc


def make_in_maps(x, W1, b1, W2, b2, W3, b3):
    """Host-side shard + layout prep. Returns one input map per core."""
    xb = np.asarray(x, dtype=np.float32).reshape(M, B, D_IN)
    W1 = np.asarray(W1, dtype=np.float32)
    W2 = np.asarray(W2, dtype=np.float32)
    W3 = np.asarray(W3, dtype=np.float32)
    b1 = np.asarray(b1, dtype=np.float32)
    b2 = np.asarray(b2, dtype=np.float32)
    b3 = np.asarray(b3, dtype=np.float32)

    in_maps = []
    for core in range(N_CORES):
        sl = slice(core * MPC, (core + 1) * MPC)
        # x: [mpc,B,1024] -> i=(c,p) -> [mpc,128,KC1,B]
        xh = np.ascontiguousarray(
            xb[sl].reshape(MPC, B, KC1, 128).transpose(0, 3, 2, 1))
        # W1: [mpc,256,1024] -> [mpc,128,KC1,256]
        w1h = np.ascontiguousarray(
            W1[sl].reshape(MPC, H1, KC1, 128).transpose(0, 3, 2, 1))
        # W2: [mpc,128,256] -> [mpc,128,KC2,128]
        w2h = np.ascontiguousarray(
            W2[sl].reshape(MPC, H2, KC2, 128).transpose(0, 3, 2, 1))
        # W3: [mpc,16,128] -> [mpc,128,16]
        w3h = np.ascontiguousarray(W3[sl].transpose(0, 2, 1))
        b1t = np.ascontiguousarray(b1[sl].reshape(MPC, OC1, 128).transpose(0, 2, 1))
        b2t = np.ascontiguousarray(b2[sl].reshape(MPC, 128, 1))
        b3t = np.ascontiguousarray(b3[sl].reshape(MPC, Z, 1))
        in_maps.append({
            "xh": xh, "w1h": w1h, "b1h": b1t,
            "w2h": w2h, "b2h": b2t, "w3h": w3h, "b3h": b3t,
        })
    return in_maps


def kernel(x, W1, b1, W2, b2, W3, b3):
    global _cached, last_results
    if _cached is None:
        _cached = build_bass()
    nc = _cached

    in_maps = make_in_maps(x, W1, b1, W2, b2, W3, b3)
    res = run_bass_kernel_spmd(nc, in_maps, list(range(N_CORES)))
    last_results = res

    # outh per core: [MPC, Z, B] -> full output [M, B, Z]
    parts = [r["outh"] for r in res.results]
    out_t = np.concatenate(parts, axis=0)             # [M, Z, B]
    return np.ascontiguousarray(out_t.transpose(0, 2, 1)).astype(np.float32)


# revision 20
# speedup vs baseline: 4.1606x; 1.0049x over previous
"""Trainium2 Bass kernel: 16-member MLP ensemble (1024 -> 256 relu -> 128 relu -> 16 tanh).

Sharding: expert-parallel over the ensemble axis -- 2 members per NeuronCore x 8 cores,
fully independent (no collectives).

Device layout strategy: the PE contracts along the partition dim, so every operand is
pre-arranged host-side with the contraction dim on partitions:
  x   -> [mpc, 128, 8, B]   (x^T tiles: input-feature chunks on partitions)
  W1  -> [mpc, 128, 8, 256] (W1^T: lhsT tiles [K=128, M=256])
  W2  -> [mpc, 128, 2, 128]
  W3  -> [mpc, 128, 16]
Hidden activations stay in SBUF between layers (h1/h2 never touch HBM); the kernel output
is out^T [mpc, 16, B] per core, un-transposed on host.

Matmuls run as float32r (fp32 data, full-rate PE mode for moving-dim >= 256).
"""

import numpy as np

import concourse.bacc as bacc
import concourse.bass as bass
import concourse.mybir as mybir
import concourse.tile as tile
from concourse.bass_utils import run_bass_kernel_spmd
from concourse.tile import add_dep_helper

M, B, Z = 16, 4096, 16
N_CORES = 8
MPC = M // N_CORES          # models per core
D_IN, H1, H2 = 1024, 256, 128
BT = 512                    # batch tile (fp32 moving-operand max / one PSUM bank)
NBT = B // BT
KC1 = D_IN // 128           # contraction chunks, layer 1
KC2 = H1 // 128             # contraction chunks, layer 2
OC1 = H1 // 128             # output chunks, layer 1

F32 = mybir.dt.float32
F32R = mybir.dt.float32r
AF = mybir.ActivationFunctionType

_cached = None
last_results = None         # BassKernelResults from the most recent run (for test harness)


def build_bass():
    nc = bacc.Bacc("TRN2", target_bir_lowering=False, debug=False, num_devices=N_CORES)

    xh = nc.dram_tensor("xh", [MPC, 128, KC1, B], F32R, kind="ExternalInput")
    w1h = nc.dram_tensor("w1h", [MPC, 128, KC1, H1], F32R, kind="ExternalInput")
    b1h = nc.dram_tensor("b1h", [MPC, 128, OC1], F32, kind="ExternalInput")
    w2h = nc.dram_tensor("w2h", [MPC, 128, KC2, H2], F32R, kind="ExternalInput")
    b2h = nc.dram_tensor("b2h", [MPC, 128, 1], F32, kind="ExternalInput")
    w3h = nc.dram_tensor("w3h", [MPC, 128, Z], F32R, kind="ExternalInput")
    b3h = nc.dram_tensor("b3h", [MPC, Z, 1], F32, kind="ExternalInput")
    outh = nc.dram_tensor("outh", [MPC, Z, B], F32, kind="ExternalOutput")

    with tile.TileContext(nc) as tc:
        with (
            tc.tile_pool(name="weights", bufs=1) as wp,
            tc.tile_pool(name="xin", bufs=5) as xp,
            tc.tile_pool(name="hid", bufs=4) as hp,
            tc.tile_pool(name="outs", bufs=4) as op,
            tc.tile_pool(name="ps", bufs=2, space="PSUM") as pp,
            tc.tile_pool(name="ps1p", bufs=4, space="PSUM") as pp1,
            tc.tile_pool(name="ps3p", bufs=1, space="PSUM") as pp3,
            tc.tile_pool(name="warm", bufs=1, space="PSUM") as wpp,
        ):
            # Weight/bias DMAs. w1 of model 0 goes first so the PE can start
            # layer 1 as early as possible; everything else trickles in behind
            # the first x tile on the queue.
            wt = [[None] * 6 for _ in range(MPC)]
            wdmas = []
            w1_0 = wp.tile([128, KC1, H1], F32R, name="w1_0", tag="w1_0")
            wdmas.append(nc.sync.dma_start(w1_0[:], w1h[0]))
            wt[0][0] = w1_0
            for m in range(MPC):
                if m > 0:
                    w1m = wp.tile([128, KC1, H1], F32R, name=f"w1_{m}", tag=f"w1_{m}")
                    wdmas.append(nc.sync.dma_start(w1m[:], w1h[m]))
                    wt[m][0] = w1m
                w2 = wp.tile([128, KC2, H2], F32R, name=f"w2_{m}", tag=f"w2_{m}")
                wdmas.append(nc.sync.dma_start(w2[:], w2h[m]))
                w3 = wp.tile([128, Z], F32R, name=f"w3_{m}", tag=f"w3_{m}")
                wdmas.append(nc.sync.dma_start(w3[:], w3h[m]))
                b1 = wp.tile([128, OC1], F32, name=f"b1_{m}", tag=f"b1_{m}")
                wdmas.append(nc.sync.dma_start(b1[:], b1h[m]))
                b2 = wp.tile([128, 1], F32, name=f"b2_{m}", tag=f"b2_{m}")
                wdmas.append(nc.sync.dma_start(b2[:], b2h[m]))
                b3 = wp.tile([Z, 1], F32, name=f"b3_{m}", tag=f"b3_{m}")
                wdmas.append(nc.sync.dma_start(b3[:], b3h[m]))
                wt[m][1:] = [w2, w3, b1, b2, b3]

            wps = wpp.tile([128, 16], F32, name="warm_ps", tag="warm_ps")

            def emit_chunk(m, tag, w1, w2, w3, b1, b2, b3, xt, xs, outs, width):
                """One fused 3-layer pass over `width` batch columns.
                xt[:, c, xs] supplies the layer-1 rhs; result stored to outh[m][:, outs]."""
                h1c = []
                for oc in range(OC1):
                    ps1 = pp1.tile([128, width], F32, name=f"ps1_{tag}_{oc}", tag="ps1")
                    for c in range(KC1):
                        nc.tensor.matmul(
                            ps1[:],
                            lhsT=w1[:, c, oc * 128:(oc + 1) * 128],
                            rhs=xt[:, c, xs],
                            start=(c == 0),
                            stop=(c == KC1 - 1),
                        )
                    h1 = hp.tile([128, width], F32R, name=f"h1_{tag}_{oc}", tag="h1")
                    nc.scalar.activation(h1[:], ps1[:], AF.Relu, bias=b1[:, oc:oc + 1])
                    h1c.append(h1)

                ps2 = pp.tile([128, width], F32, name=f"ps2_{tag}", tag="ps2")
                for c in range(KC2):
                    nc.tensor.matmul(
                        ps2[:],
                        lhsT=w2[:, c, :],
                        rhs=h1c[c][:],
                        start=(c == 0),
                        stop=(c == KC2 - 1),
                    )
                h2 = hp.tile([128, width], F32R, name=f"h2_{tag}", tag="h2")
                nc.scalar.activation(h2[:], ps2[:], AF.Relu, bias=b2[:, 0:1])

                ps3 = pp3.tile([Z, width], F32, name=f"ps3_{tag}", tag="ps3")
                nc.tensor.matmul(ps3[:], lhsT=w3[:], rhs=h2[:], start=True, stop=True)
                ot = op.tile([Z, width], F32, name=f"ot_{tag}", tag="ot")
                nc.scalar.activation(ot[:], ps3[:], AF.Tanh, bias=b3[:, 0:1])
                store_eng = nc.scalar if width != BT else nc.gpsimd
                store_eng.dma_start(outh[m][:, outs], ot[:])

            XW = BT               # columns per x DMA (2 MiB transfers)
            for m in range(MPC):
                w1, w2, w3, b1, b2, b3 = wt[m]
                # Weight-touch warmups, per model: the walrus fp32r self-loading
                # matmul has a single sync-wait slot, so no real matmul may wait
                # on both its weight DMA and its rhs producer. Touch each weight
                # tile with a tiny matmul carrying the weight-DMA wait alone.
                nc.tensor.matmul(wps[:], lhsT=w1[:, 0, 0:128],
                                 rhs=w1[:, 0, 0:16], start=True, stop=True)
                nc.tensor.matmul(wps[:], lhsT=w2[:, 0, 0:128],
                                 rhs=w2[:, 0, 0:16], start=True, stop=True)
                nc.tensor.matmul(wps[0:16, :], lhsT=w3[:, 0:16],
                                 rhs=w3[:, 0:16], start=True, stop=True)

                last = MPC - 1 == m
                for tx in range(B // XW):
                    xt = xp.tile([128, KC1, XW], F32R, name=f"x_{m}_{tx}", tag="xt")
                    x_ap = xh[m][:, :, tx * XW:(tx + 1) * XW]
                    tail = last and tx == B // XW - 1
                    if not tail:
                        xdma = nc.sync.dma_start(xt[:], x_ap)
                        if m == 0 and tx == 0:
                            # Keeps the first bulk x chunk at the queue head with
                            # the small weight DMAs immediately behind it.
                            for wd in wdmas:
                                add_dep_helper(wd.ins, xdma.ins, sync=False,
                                               reason="weights before x bulk stream")
                        emit_chunk(m, f"{m}_{tx}", w1, w2, w3, b1, b2, b3,
                                   xt, slice(0, XW), slice(tx * XW, (tx + 1) * XW), XW)
                    else:
                        # Final chunk: split into halves so the tail drain
                        # overlaps the last x bytes still in flight.
                        hw_ = XW // 2
                        for h in range(2):
                            hs = slice(h * hw_, (h + 1) * hw_)
                            # split each half k-wise too: layer-1 accumulation of
                            # chunks 0-3 starts while chunks 4-7 are in flight
                            nc.sync.dma_start(xt[:, 0:KC1 // 2, hs],
                                              x_ap[:, 0:KC1 // 2, hs])
                            nc.sync.dma_start(xt[:, KC1 // 2:KC1, hs],
                                              x_ap[:, KC1 // 2:KC1, hs])
                            emit_chunk(m, f"{m}_{tx}_h{h}", w1, w2, w3, b1, b2, b3,
                                       xt, slice(h * hw_, (h + 1) * hw_),
                                       slice(tx * XW + h * hw_, tx * XW + (h + 1) * hw_), hw_)

    nc.compile()
    return nc


def make_in_maps(x, W1, b1, W2, b2, W3, b3):
    """Host-side shard + layout prep. Returns one input map per core."""
    xb = np.asarray(x, dtype=np.float32).reshape(M, B, D_IN)
    W1 = np.asarray(W1, dtype=np.float32)
    W2 = np.asarray(W2, dtype=np.float32)
    W3 = np.asarray(W3, dtype=np.float32)
    b1 = np.asarray(b1, dtype=np.float32)
    b2 = np.asarray(b2, dtype=np.float32)
    b3 = np.asarray(b3, dtype=np.float32)

    in_maps = []
    for core in range(N_CORES):
        sl = slice(core * MPC, (core + 1) * MPC)
        # x: [mpc,B,1024] -> i=(c,p) -> [mpc,128,KC1,B]
        xh = np.ascontiguousarray(
            xb[sl].reshape(MPC, B, KC1, 128).transpose(0, 3, 2, 1))
        # W1: [mpc,256,1024] -> [mpc,128,KC1,256]
        w1h = np.ascontiguousarray(
            W1[sl].reshape(MPC, H1, KC1, 128).transpose(0, 3, 2, 1))
        # W2: [mpc,128,256] -> [mpc,128,KC2,128]
        w2h = np.ascontiguousarray(
            W2[sl].reshape(MPC, H2, KC2, 128).transpose(0, 3, 2, 1))
        # W3: [mpc,16,128] -> [mpc,128,16]
        w3h = np.ascontiguousarray(W3[sl].transpose(0, 2, 1))
        b1t = np.ascontiguousarray(b1[sl].reshape(MPC, OC1, 128).transpose(0, 2, 1))
        b2t = np.ascontiguousarray(b2[sl].reshape(MPC, 128, 1))
        b3t = np.ascontiguousarray(b3[sl].reshape(MPC, Z, 1))
        in_maps.append({
            "xh": xh, "w1h": w1h, "b1h": b1t,
            "w2h": w2h, "b2h": b2t, "w3h": w3h, "b3h": b3t,
        })
    return in_maps


def kernel(x, W1, b1, W2, b2, W3, b3):
    global _cached, last_results
    if _cached is None:
        _cached = build_bass()
    nc = _cached

    in_maps = make_in_maps(x, W1, b1, W2, b2, W3, b3)
    res = run_bass_kernel_spmd(nc, in_maps, list(range(N_CORES)))
    last_results = res

    # outh per core: [MPC, Z, B] -> full output [M, B, Z]
    parts = [r["outh"] for r in res.results]
    out_t = np.concatenate(parts, axis=0)             # [M, Z, B]
    return np.ascontiguousarray(out_t.transpose(0, 2, 1)).astype(np.float32)


# revision 24
# speedup vs baseline: 5.4801x; 1.3172x over previous
"""Trainium2 Bass kernel: 16-member MLP ensemble (1024 -> 256 relu -> 128 relu -> 16 tanh).

Sharding: expert-parallel over the ensemble axis -- 2 members per NeuronCore x 8 cores,
fully independent (no collectives).

Device layout strategy: the PE contracts along the partition dim, so every operand is
pre-arranged host-side with the contraction dim on partitions:
  x   -> [mpc, 128, 8, B]   (x^T tiles: input-feature chunks on partitions)
  W1  -> [mpc, 128, 8, 256] (W1^T: lhsT tiles [K=128, M=256])
  W2  -> [mpc, 128, 2, 128]
  W3  -> [mpc, 128, 16]
Hidden activations stay in SBUF between layers (h1/h2 never touch HBM); the kernel output
is out^T [mpc, 16, B] per core, un-transposed on host.

Matmuls run as float32r (fp32 data, full-rate PE mode for moving-dim >= 256).
"""

import numpy as np

import concourse.bacc as bacc
import concourse.bass as bass
import concourse.mybir as mybir
import concourse.tile as tile
from concourse.bass_utils import run_bass_kernel_spmd
from concourse.tile import add_dep_helper

M, B, Z = 16, 4096, 16
N_CORES = 8
MPC = M // N_CORES          # models per core
D_IN, H1, H2 = 1024, 256, 128
BT = 512                    # batch tile (fp32 moving-operand max / one PSUM bank)
NBT = B // BT
KC1 = D_IN // 128           # contraction chunks, layer 1
KC2 = H1 // 128             # contraction chunks, layer 2
OC1 = H1 // 128             # output chunks, layer 1

F32 = mybir.dt.float32
F32R = mybir.dt.float32r
F16 = mybir.dt.float16
AF = mybir.ActivationFunctionType

_cached = None
last_results = None         # BassKernelResults from the most recent run (for test harness)


def build_bass():
    nc = bacc.Bacc("TRN2", target_bir_lowering=False, debug=False, num_devices=N_CORES)

    xh = nc.dram_tensor("xh", [MPC, 128, KC1, B], F16, kind="ExternalInput")
    w1h = nc.dram_tensor("w1h", [MPC, 128, KC1, H1], F16, kind="ExternalInput")
    b1h = nc.dram_tensor("b1h", [MPC, 128, OC1], F32, kind="ExternalInput")
    w2h = nc.dram_tensor("w2h", [MPC, 128, KC2, H2], F32R, kind="ExternalInput")
    b2h = nc.dram_tensor("b2h", [MPC, 128, 1], F32, kind="ExternalInput")
    w3h = nc.dram_tensor("w3h", [MPC, 128, Z], F32R, kind="ExternalInput")
    b3h = nc.dram_tensor("b3h", [MPC, Z, 1], F32, kind="ExternalInput")
    outh = nc.dram_tensor("outh", [MPC, Z, B], F32, kind="ExternalOutput")

    with tile.TileContext(nc) as tc:
        with (
            tc.tile_pool(name="weights", bufs=1) as wp,
            tc.tile_pool(name="xin", bufs=5) as xp,
            tc.tile_pool(name="hid", bufs=4) as hp,
            tc.tile_pool(name="outs", bufs=4) as op,
            tc.tile_pool(name="ps", bufs=2, space="PSUM") as pp,
            tc.tile_pool(name="ps1p", bufs=4, space="PSUM") as pp1,
            tc.tile_pool(name="ps3p", bufs=1, space="PSUM") as pp3,
            tc.tile_pool(name="warm", bufs=1, space="PSUM") as wpp,
        ):
            # Weight/bias DMAs. w1 of model 0 goes first so the PE can start
            # layer 1 as early as possible; everything else trickles in behind
            # the first x tile on the queue.
            wt = [[None] * 6 for _ in range(MPC)]
            wdmas = []
            w1_0 = wp.tile([128, KC1, H1], F16, name="w1_0", tag="w1_0")
            wdmas.append(nc.sync.dma_start(w1_0[:], w1h[0]))
            wt[0][0] = w1_0
            for m in range(MPC):
                if m > 0:
                    w1m = wp.tile([128, KC1, H1], F16, name=f"w1_{m}", tag=f"w1_{m}")
                    wdmas.append(nc.sync.dma_start(w1m[:], w1h[m]))
                    wt[m][0] = w1m
                w2 = wp.tile([128, KC2, H2], F32R, name=f"w2_{m}", tag=f"w2_{m}")
                wdmas.append(nc.sync.dma_start(w2[:], w2h[m]))
                w3 = wp.tile([128, Z], F32R, name=f"w3_{m}", tag=f"w3_{m}")
                wdmas.append(nc.sync.dma_start(w3[:], w3h[m]))
                b1 = wp.tile([128, OC1], F32, name=f"b1_{m}", tag=f"b1_{m}")
                wdmas.append(nc.sync.dma_start(b1[:], b1h[m]))
                b2 = wp.tile([128, 1], F32, name=f"b2_{m}", tag=f"b2_{m}")
                wdmas.append(nc.sync.dma_start(b2[:], b2h[m]))
                b3 = wp.tile([Z, 1], F32, name=f"b3_{m}", tag=f"b3_{m}")
                wdmas.append(nc.sync.dma_start(b3[:], b3h[m]))
                wt[m][1:] = [w2, w3, b1, b2, b3]

            wps = wpp.tile([128, 16], F32, name="warm_ps", tag="warm_ps")

            def emit_chunk(m, tag, w1, w2, w3, b1, b2, b3, xt, xs, outs, width):
                """One fused 3-layer pass over `width` batch columns.
                xt[:, c, xs] supplies the layer-1 rhs; result stored to outh[m][:, outs]."""
                h1c = []
                for oc in range(OC1):
                    ps1 = pp1.tile([128, width], F32, name=f"ps1_{tag}_{oc}", tag="ps1")
                    for c in range(KC1):
                        nc.tensor.matmul(
                            ps1[:],
                            lhsT=w1[:, c, oc * 128:(oc + 1) * 128],
                            rhs=xt[:, c, xs],
                            start=(c == 0),
                            stop=(c == KC1 - 1),
                        )
                    h1 = hp.tile([128, width], F32R, name=f"h1_{tag}_{oc}", tag="h1")
                    nc.scalar.activation(h1[:], ps1[:], AF.Relu, bias=b1[:, oc:oc + 1])
                    h1c.append(h1)

                ps2 = pp.tile([128, width], F32, name=f"ps2_{tag}", tag="ps2")
                for c in range(KC2):
                    nc.tensor.matmul(
                        ps2[:],
                        lhsT=w2[:, c, :],
                        rhs=h1c[c][:],
                        start=(c == 0),
                        stop=(c == KC2 - 1),
                    )
                h2 = hp.tile([128, width], F32R, name=f"h2_{tag}", tag="h2")
                nc.scalar.activation(h2[:], ps2[:], AF.Relu, bias=b2[:, 0:1])

                ps3 = pp3.tile([Z, width], F32, name=f"ps3_{tag}", tag="ps3")
                nc.tensor.matmul(ps3[:], lhsT=w3[:], rhs=h2[:], start=True, stop=True)
                ot = op.tile([Z, width], F32, name=f"ot_{tag}", tag="ot")
                nc.scalar.activation(ot[:], ps3[:], AF.Tanh, bias=b3[:, 0:1])
                store_eng = nc.scalar if width != BT else nc.gpsimd
                store_eng.dma_start(outh[m][:, outs], ot[:])

            XW = BT               # columns per x DMA (2 MiB transfers)
            for m in range(MPC):
                w1, w2, w3, b1, b2, b3 = wt[m]
                # Weight-touch warmups, per model: the walrus fp32r self-loading
                # matmul has a single sync-wait slot, so no real matmul may wait
                # on both its weight DMA and its rhs producer. Touch each weight
                # tile with a tiny matmul carrying the weight-DMA wait alone.
                nc.tensor.matmul(wps[:], lhsT=w1[:, 0, 0:128],
                                 rhs=w1[:, 0, 0:16], start=True, stop=True)
                nc.tensor.matmul(wps[:], lhsT=w2[:, 0, 0:128],
                                 rhs=w2[:, 0, 0:16], start=True, stop=True)
                nc.tensor.matmul(wps[0:16, :], lhsT=w3[:, 0:16],
                                 rhs=w3[:, 0:16], start=True, stop=True)

                last = MPC - 1 == m
                for tx in range(B // XW):
                    xt = xp.tile([128, KC1, XW], F16, name=f"x_{m}_{tx}", tag="xt")
                    x_ap = xh[m][:, :, tx * XW:(tx + 1) * XW]
                    tail = last and tx == B // XW - 1
                    if not tail:
                        xdma = nc.sync.dma_start(xt[:], x_ap)
                        if m == 0 and tx == 0:
                            # Keeps the first bulk x chunk at the queue head with
                            # the small weight DMAs immediately behind it.
                            for wd in wdmas:
                                add_dep_helper(wd.ins, xdma.ins, sync=False,
                                               reason="weights before x bulk stream")
                        emit_chunk(m, f"{m}_{tx}", w1, w2, w3, b1, b2, b3,
                                   xt, slice(0, XW), slice(tx * XW, (tx + 1) * XW), XW)
                    else:
                        # Final chunk: split into halves so the tail drain
                        # overlaps the last x bytes still in flight.
                        hw_ = XW // 2
                        for h in range(2):
                            hs = slice(h * hw_, (h + 1) * hw_)
                            # split each half k-wise too: layer-1 accumulation of
                            # chunks 0-3 starts while chunks 4-7 are in flight
                            nc.sync.dma_start(xt[:, 0:KC1 // 2, hs],
                                              x_ap[:, 0:KC1 // 2, hs])
                            nc.sync.dma_start(xt[:, KC1 // 2:KC1, hs],
                                              x_ap[:, KC1 // 2:KC1, hs])
                            emit_chunk(m, f"{m}_{tx}_h{h}", w1, w2, w3, b1, b2, b3,
                                       xt, slice(h * hw_, (h + 1) * hw_),
                                       slice(tx * XW + h * hw_, tx * XW + (h + 1) * hw_), hw_)

    nc.compile()
    return nc


def make_in_maps(x, W1, b1, W2, b2, W3, b3):
    """Host-side shard + layout prep. Returns one input map per core."""
    xb = np.asarray(x, dtype=np.float32).reshape(M, B, D_IN)
    W1 = np.asarray(W1, dtype=np.float32)
    W2 = np.asarray(W2, dtype=np.float32)
    W3 = np.asarray(W3, dtype=np.float32)
    b1 = np.asarray(b1, dtype=np.float32)
    b2 = np.asarray(b2, dtype=np.float32)
    b3 = np.asarray(b3, dtype=np.float32)

    in_maps = []
    for core in range(N_CORES):
        sl = slice(core * MPC, (core + 1) * MPC)
        # x: [mpc,B,1024] -> i=(c,p) -> [mpc,128,KC1,B]
        xh = np.ascontiguousarray(
            xb[sl].reshape(MPC, B, KC1, 128).transpose(0, 3, 2, 1)).astype(np.float16)
        # W1: [mpc,256,1024] -> [mpc,128,KC1,256]
        w1h = np.ascontiguousarray(
            W1[sl].reshape(MPC, H1, KC1, 128).transpose(0, 3, 2, 1)).astype(np.float16)
        # W2: [mpc,128,256] -> [mpc,128,KC2,128]
        w2h = np.ascontiguousarray(
            W2[sl].reshape(MPC, H2, KC2, 128).transpose(0, 3, 2, 1))
        # W3: [mpc,16,128] -> [mpc,128,16]
        w3h = np.ascontiguousarray(W3[sl].transpose(0, 2, 1))
        b1t = np.ascontiguousarray(b1[sl].reshape(MPC, OC1, 128).transpose(0, 2, 1))
        b2t = np.ascontiguousarray(b2[sl].reshape(MPC, 128, 1))
        b3t = np.ascontiguousarray(b3[sl].reshape(MPC, Z, 1))
        in_maps.append({
            "xh": xh, "w1h": w1h, "b1h": b1t,
            "w2h": w2h, "b2h": b2t, "w3h": w3h, "b3h": b3t,
        })
    return in_maps


def kernel(x, W1, b1, W2, b2, W3, b3):
    global _cached, last_results
    if _cached is None:
        _cached = build_bass()
    nc = _cached

    in_maps = make_in_maps(x, W1, b1, W2, b2, W3, b3)
    res = run_bass_kernel_spmd(nc, in_maps, list(range(N_CORES)))
    last_results = res

    # outh per core: [MPC, Z, B] -> full output [M, B, Z]
    parts = [r["outh"] for r in res.results]
    out_t = np.concatenate(parts, axis=0)             # [M, Z, B]
    return np.ascontiguousarray(out_t.transpose(0, 2, 1)).astype(np.float32)
